# revision 1
# baseline (speedup 1.0000x reference)
"""Trainium2 Bass kernel for nn_ADV_75574244541027 (gumbel-augmentation loss).

Self-contained: hardcodes shapes/sharding; builds one SPMD Bass program,
runs it on 8 NeuronCores via run_bass_kernel_spmd, returns the 5-scalar
output [aug_loss, mi, reg_aug, add, drop].

Sharding: data-parallel over batch B for the elementwise/reg part
(x/prob/u split into 8 row blocks of 256), row-sharded similarity matrix
for calc_I (each core computes its 1024x8192 block of exp(cos/T)); the
per-shard column sums + diagonals + row sums + reg partial sums travel in
a single AllGather, after which every core computes the full (identical)
scalar output.
"""

import sys

for _p in ("/opt/trn_rl_repo", "/root/.axon_site/_ro/trn_rl_repo"):
    if _p not in sys.path:
        sys.path.insert(0, _p)

import numpy as np
import ml_dtypes

import concourse.bass as bass
import concourse.mybir as mybir
import concourse.bacc as bacc
import concourse.tile as tile
import concourse.bass_isa as bass_isa
from concourse.bass_utils import run_bass_kernel_spmd

# ---------------- problem constants ----------------
NCORES = 8
B, N, K, D = 2048, 20000, 4, 64
BIAS = 1e-4
C1 = 1.0 - 2.0 * BIAS          # (1 - 2*bias)
S5 = 5.0                       # 1/TAU == 1/TEMP == 5
P = 128
BROWS = B // NCORES            # 256 batch rows per core
FREE = BROWS * N // P          # 40000 free-dim elements per partition
FT = 1250                      # part-A tile width
NHALF = 8                      # part-A phase halves (bounds w buffer)
NT = FREE // FT                # 20 tiles total
NTH = NT // NHALF              # tiles per half
R = K * BROWS                  # 1024 sim-matrix rows per core
RB = R // P                    # 8 row blocks
CBW = 1024                     # part-B column block width
KB = K * B                     # 8192 sim-matrix columns
NCB = KB // CBW                # 8 column blocks
PAY = KB + R + R + 8           # payload: colsum | pos | rowsum | st,s1,sx,pad
BN = float(B) * float(N)

f32 = mybir.dt.float32
bf16 = mybir.dt.bfloat16
FT_ = mybir.ActivationFunctionType
ALU = mybir.AluOpType
AXIS = mybir.AxisListType
RED = bass_isa.ReduceOp


def build():
    nc = bacc.Bacc("TRN2", target_bir_lowering=False, debug=False,
                   num_devices=NCORES)

    x_d = nc.dram_tensor("xb", [P, FREE], bf16, kind="ExternalInput")
    u_d = nc.dram_tensor("u", [P, FREE], f32, kind="ExternalInput")
    p_d = nc.dram_tensor("prob", [P, FREE], f32, kind="ExternalInput")
    z1t_d = nc.dram_tensor("z1t", [D, R], f32, kind="ExternalInput")
    z2t_d = nc.dram_tensor("z2t", [D, KB], f32, kind="ExternalInput")
    z2ts_d = nc.dram_tensor("z2ts", [D, R], f32, kind="ExternalInput")
    out_d = nc.dram_tensor("out", [1, 8], f32, kind="ExternalOutput")

    PAY1 = KB + R + R
    pay_d = nc.dram_tensor("pay", [1, PAY1], f32)
    ag_d = nc.dram_tensor("ag", [NCORES, PAY1], f32, addr_space="Shared")
    pay2_d = nc.dram_tensor("pay2", [1, 8], f32)
    ag2_d = nc.dram_tensor("ag2", [NCORES, 8], f32, addr_space="Shared")
    bn1_d = nc.dram_tensor("bn1", [R], f32)
    bn2_d = nc.dram_tensor("bn2", [KB], f32)
    bn2b_d = nc.dram_tensor("bn2b", [KB], f32)
    bns_d = nc.dram_tensor("bns", [R], f32)
    bdot_d = nc.dram_tensor("bdot", [R], f32)
    posg_d = nc.dram_tensor("posg", [KB], f32)
    s1g_d = nc.dram_tensor("s1g", [KB], f32)

    nrm_acts = []
    exp_instrs = []
    ln_h = [[] for _ in range(NHALF)]
    sig_h = [[] for _ in range(NHALF)]
    tail_lns = []
    pe_norm = []
    dve_pre = []
    dve_mid = []
    dve_col = []
    dve_L = [[] for _ in range(NHALF)]
    dve_S = [[] for _ in range(NHALF)]
    pe_S_cb = []
    pe_C_cb = []
    pe_phaseS = []
    pe_tail = []

    with tile.TileContext(nc) as tc:
        from contextlib import ExitStack
        es = ExitStack()
        with es:
            # every pool stays open for the whole kernel: no SBUF reuse
            # across pools -> no release->alloc dependencies -> the ACT
            # chain can be ordered freely.
            persist = es.enter_context(tc.tile_pool(name="persist", bufs=1))

            ones = persist.tile([P, 1], f32, tag="ones")
            nc.gpsimd.memset(ones[:], 1.0)
            ones_b = persist.tile([P, 1], bf16, tag="ones_b")
            nc.gpsimd.memset(ones_b[:], 1.0)
            onesD = persist.tile([D, 1], f32, tag="onesD")
            nc.gpsimd.memset(onesD[:], 1.0)
            cb_hi = persist.tile([P, 1], f32, tag="cb_hi")
            nc.gpsimd.memset(cb_hi[:], 1.0 - BIAS)
            cb_lo = persist.tile([P, 1], f32, tag="cb_lo")
            nc.gpsimd.memset(cb_lo[:], BIAS)
            cb_bn = persist.tile([P, 1], f32, tag="cb_bn")
            nc.gpsimd.memset(cb_bn[:], BN)
            cb_m20 = persist.tile([P, 1], f32, tag="cb_m20")
            nc.gpsimd.memset(cb_m20[:], -20.0)

            st_all = persist.tile([P, NT], f32, tag="st_all")
            s1_all = persist.tile([P, NT], f32, tag="s1_all")
            scale5 = persist.tile([P, RB], f32, tag="scale5")
            rn1 = persist.tile([P, RB], f32, tag="rn1")
            rn2s = persist.tile([P, RB], f32, tag="rn2s")
            pos_sb = persist.tile([P, RB], f32, tag="pos_sb")
            rs_all = persist.tile([P, RB * NCB], f32, tag="rs_all")
            rs = persist.tile([P, RB], f32, tag="rs")

            # ---- part B operands (permanent) ----
            pb = es.enter_context(tc.tile_pool(name="pb", bufs=1))
            z1t_sb = pb.tile([D, R], f32, tag="z1t")
            nc.gpsimd.dma_start(z1t_sb[:], z1t_d[:, :])
            z1tb = pb.tile([D, R], bf16, tag="z1tb")
            nc.vector.tensor_copy(z1tb[:], z1t_sb[:])
            z2nt = pb.tile([D, KB], bf16, tag="z2nt")
            z2ts_sb = pb.tile([D, R], f32, tag="z2ts")
            nc.gpsimd.dma_start(z2ts_sb[:], z2ts_d[:, :])

            psS = es.enter_context(
                tc.tile_pool(name="psS", bufs=2, space="PSUM"))
            psC = es.enter_context(
                tc.tile_pool(name="psC", bufs=1, space="PSUM"))
            accp = es.enter_context(
                tc.tile_pool(name="accp", bufs=1, space="PSUM"))

            def psum_colsums(dst_row, sq_tile, width):
                for c in range(0, width, 512):
                    pr = psS.tile([1, 512], f32, tag="S2")
                    pe_norm.append(nc.tensor.matmul(
                        pr[0:1, :], onesD[:, 0:1], sq_tile[:, c:c + 512],
                        start=True, stop=True))
                    dve_pre.append(nc.vector.tensor_copy(
                        dst_row[0:1, c:c + 512], pr[0:1, :]))

            QW = KB // 4
            zn = es.enter_context(tc.tile_pool(name="znorm", bufs=1))
            # ---- z1 / z2-shard norms, diag -> pos ----
            sq1 = zn.tile([D, R], f32, tag="sq1")
            dve_pre.append(nc.vector.tensor_mul(sq1[:], z1t_sb[:], z1t_sb[:]))
            n21 = zn.tile([1, R], f32, tag="n21")
            psum_colsums(n21, sq1, R)
            nc.gpsimd.dma_start(bn1_d[:], n21[0:1, :])
            nc.gpsimd.dma_start(
                rn1[:], bn1_d.ap().rearrange("(c p) -> p c", p=P))
            nrm_acts.append(nc.scalar.activation(rn1[:], rn1[:], FT_.Ln))
            nrm_acts.append(nc.scalar.activation(
                rn1[:], rn1[:], FT_.Exp, scale=-0.5))
            dve_mid.append(nc.vector.tensor_scalar_mul(scale5[:], rn1[:], S5))

            sqs = zn.tile([D, R], f32, tag="sq1")
            dve_pre.append(nc.vector.tensor_mul(sqs[:], z2ts_sb[:], z2ts_sb[:]))
            n2sr = zn.tile([1, R], f32, tag="n21")
            psum_colsums(n2sr, sqs, R)
            nc.gpsimd.dma_start(bns_d[:], n2sr[0:1, :])
            nc.gpsimd.dma_start(
                rn2s[:], bns_d.ap().rearrange("(c p) -> p c", p=P))
            nrm_acts.append(nc.scalar.activation(rn2s[:], rn2s[:], FT_.Ln))
            nrm_acts.append(nc.scalar.activation(
                rn2s[:], rn2s[:], FT_.Exp, scale=-0.5))

            qd = zn.tile([D, R], f32, tag="sq1")
            dve_pre.append(nc.vector.tensor_mul(qd[:], z1t_sb[:], z2ts_sb[:]))
            dotr = zn.tile([1, R], f32, tag="n21")
            psum_colsums(dotr, qd, R)
            nc.gpsimd.dma_start(bdot_d[:], dotr[0:1, :])
            dots = persist.tile([P, RB], f32, tag="dots")
            nc.gpsimd.dma_start(
                dots[:], bdot_d.ap().rearrange("(c p) -> p c", p=P))
            m1 = persist.tile([P, RB], f32, tag="m1")
            dve_mid.append(nc.vector.tensor_mul(m1[:], dots[:], rn1[:]))
            m2 = persist.tile([P, RB], f32, tag="m2")
            dve_mid.append(nc.vector.tensor_mul(m2[:], m1[:], rn2s[:]))
            exp_instrs.append(nc.scalar.activation(
                pos_sb[:], m2[:], FT_.Exp, scale=S5))

            # ---- normalize z2: stage loops so quarters pipeline ----
            nbqs = []
            for q in range(4):
                z2q = zn.tile([D, QW], f32, tag="z2q")
                nc.sync.dma_start(z2q[:], z2t_d[:, q * QW:(q + 1) * QW])
                sqq = zn.tile([D, QW], f32, tag="sqq")
                dve_pre.append(nc.vector.tensor_mul(sqq[:], z2q[:], z2q[:]))
                for hh in range(2):
                    nrow = zn.tile([1, QW // 2], f32, tag="n21")
                    psum_colsums(nrow, sqq[:, hh * (QW // 2):
                                           (hh + 1) * (QW // 2)], QW // 2)
                    nc.gpsimd.dma_start(
                        bn2_d[q * QW + hh * (QW // 2):
                              q * QW + (hh + 1) * (QW // 2)], nrow[0:1, :])
                nbq = persist.tile([P, QW // P], f32, tag=f"nbq{q}")
                nc.gpsimd.dma_start(
                    nbq[:], bn2_d[q * QW:(q + 1) * QW].rearrange(
                        "(c p) -> p c", p=P))
                nbqs.append(nbq)
            for q in range(4):
                nbq = nbqs[q]
                nrm_acts.append(nc.scalar.activation(nbq[:], nbq[:], FT_.Ln))
                nrm_acts.append(nc.scalar.activation(
                    nbq[:], nbq[:], FT_.Exp, scale=-0.5))
                nc.gpsimd.dma_start(
                    bn2b_d[q * QW:(q + 1) * QW].rearrange(
                        "(c p) -> p c", p=P), nbq[:])
            for q in range(4):
                bcq = zn.tile([D, QW], f32, tag="bcq")
                nc.gpsimd.dma_start(
                    bcq[:],
                    bn2b_d[q * QW:(q + 1) * QW].rearrange(
                        "(o n) -> o n", o=1).to_broadcast((D, QW)))
                z2q2 = zn.tile([D, QW], f32, tag="z2q2")
                nc.sync.dma_start(z2q2[:], z2t_d[:, q * QW:(q + 1) * QW])
                dve_mid.append(nc.vector.tensor_mul(
                    z2nt[:, q * QW:(q + 1) * QW], z2q2[:], bcq[:]))

            # ---- part B main loop ----
            pbl = es.enter_context(tc.tile_pool(name="pbl", bufs=5))
            colp = es.enter_context(tc.tile_pool(name="colp", bufs=1))
            colsum_sb = colp.tile([1, KB], f32, tag="colsum")
            for cb in range(NCB):
                pe_S_cb.append([])
                pe_C_cb.append([])
                cps = psC.tile([1, CBW], f32, tag="C")
                for rb in range(RB):
                    sps = psS.tile([P, CBW], f32, tag="S2")
                    for half in range(2):
                        pe_S_cb[cb].append(nc.tensor.matmul(
                            sps[:, half * 512:(half + 1) * 512],
                            z1tb[:, rb * P:(rb + 1) * P],
                            z2nt[:, cb * CBW + half * 512:
                                 cb * CBW + (half + 1) * 512],
                            start=True, stop=True))
                    et = pbl.tile([P, CBW], bf16, tag="E")
                    exp_instrs.append(nc.scalar.activation(
                        et[:], sps[:], FT_.Exp,
                        scale=scale5[:, rb:rb + 1],
                        accum_out=rs_all[:, rb * NCB + cb:
                                         rb * NCB + cb + 1]))
                    for half in range(2):
                        pe_C_cb[cb].append(nc.tensor.matmul(
                            cps[0:1, half * 512:(half + 1) * 512],
                            ones_b[:, 0:1],
                            et[:, half * 512:(half + 1) * 512],
                            start=(rb == 0), stop=(rb == RB - 1)))
                dve_col.append(nc.vector.tensor_copy(
                    colsum_sb[0:1, cb * CBW:(cb + 1) * CBW], cps[0:1, :]))

            # ---- part A ----
            plu = es.enter_context(tc.tile_pool(name="plu", bufs=2))
            plp = es.enter_context(tc.tile_pool(name="plp", bufs=2))
            plt = es.enter_context(tc.tile_pool(name="plt", bufs=4))
            wpool = es.enter_context(tc.tile_pool(name="wpool", bufs=2))
            psx = es.enter_context(tc.tile_pool(name="psx", bufs=4))
            pscr = es.enter_context(tc.tile_pool(name="pscr", bufs=1))

            acc_sx = accp.tile([1, 512], f32, tag="acc")
            pe_sx = [[] for _ in range(NHALF)]
            for hf in range(NHALF):
                w_sb = wpool.tile([P, NTH * FT], bf16, tag="w")
                w2_sb = wpool.tile([P, NTH * FT], bf16, tag="w2")
                for ti in range(NTH):
                    t = hf * NTH + ti
                    sl = slice(t * FT, (t + 1) * FT)
                    wsl = slice(ti * FT, (ti + 1) * FT)
                    ut = plu.tile([P, FT], f32, tag="u")
                    nc.sync.dma_start(ut[:], u_d[:, sl])
                    pt = plp.tile([P, FT], f32, tag="p")
                    nc.sync.dma_start(pt[:], p_d[:, sl])
                    xt = psx.tile([P, FT], bf16, tag="x")
                    nc.sync.dma_start(xt[:], x_d[:, sl])
                    l1 = plt.tile([P, FT], f32, tag="ltmp")
                    ln_h[hf].append(nc.scalar.activation(
                        l1[:], ut[:], FT_.Ln, bias=cb_hi[:, 0:1],
                        scale=-C1))
                    l2 = plt.tile([P, FT], f32, tag="ltmp")
                    ln_h[hf].append(nc.scalar.activation(
                        l2[:], ut[:], FT_.Ln, bias=cb_lo[:, 0:1], scale=C1))
                    dt = plt.tile([P, FT], f32, tag="ltmp")
                    dve_L[hf].append(nc.vector.scalar_tensor_tensor(
                        dt[:], l1[:], -1.0, l2[:], op0=ALU.mult,
                        op1=ALU.add))
                    dve_L[hf].append(nc.vector.scalar_tensor_tensor(
                        w_sb[:, wsl], dt[:], 1.0, pt[:],
                        op0=ALU.mult, op1=ALU.subtract))
                    # w2 = (w + 4) * x  -> sigmoid(5*w2 - 20) == sigmoid(5w)*x
                    dve_L[hf].append(nc.vector.scalar_tensor_tensor(
                        w2_sb[:, wsl], w_sb[:, wsl], 4.0, xt[:],
                        op0=ALU.add, op1=ALU.mult))
                    last = (t == NT - 1)
                    for c in range(0, FT, 512):
                        cl = min(512, FT - c)
                        pe_sx[hf].append(nc.tensor.matmul(
                            acc_sx[0:1, 0:cl], ones_b[:, 0:1],
                            xt[:, c:c + cl],
                            start=(t == 0 and c == 0),
                            stop=(last and c + 512 >= FT)))
                for ti in range(NTH):
                    t = hf * NTH + ti
                    wsl = slice(ti * FT, (ti + 1) * FT)
                    sg = pscr.tile([P, FT], bf16, tag="s")
                    sig_h[hf].append(nc.scalar.activation(
                        sg[:], w_sb[:, wsl], FT_.Sigmoid, scale=S5,
                        accum_out=st_all[:, t:t + 1]))
                    sg2 = pscr.tile([P, FT], bf16, tag="s")
                    sig_h[hf].append(nc.scalar.activation(
                        sg2[:], w2_sb[:, wsl], FT_.Sigmoid, scale=S5,
                        bias=cb_m20[:, 0:1],
                        accum_out=s1_all[:, t:t + 1]))

            dve_col.append(nc.vector.tensor_reduce(
                rs[:], rs_all[:].rearrange("p (rb cb) -> p rb cb", cb=NCB),
                axis=AXIS.X, op=ALU.add))
            nc.sync.dma_start(pay_d[0:1, 0:KB], colsum_sb[0:1, :])
            nc.sync.dma_start(
                pay_d[0, KB:KB + R].rearrange("(rb p) -> p rb", p=P), rs[:])
            nc.sync.dma_start(
                pay_d[0, KB + R:KB + 2 * R].rearrange("(rb p) -> p rb", p=P),
                pos_sb[:])
            nc.gpsimd.collective_compute(
                "AllGather", ALU.bypass,
                replica_groups=[list(range(NCORES))],
                ins=[pay_d.ap().opt()], outs=[ag_d.ap().opt()])

            # ---- totals -> payload2 -> AG#2 ----
            stv = persist.tile([P, 1], f32, tag="stv")
            nc.vector.tensor_reduce(stv[:], st_all[:], axis=AXIS.X,
                                    op=ALU.add)
            s1v = persist.tile([P, 1], f32, tag="s1v")
            nc.vector.tensor_reduce(s1v[:], s1_all[:], axis=AXIS.X,
                                    op=ALU.add)
            sxrow = persist.tile([1, 512], f32, tag="sxrow")
            nc.vector.tensor_copy(sxrow[0:1, :], acc_sx[0:1, :])
            payscal = persist.tile([1, 8], f32, tag="payscal")
            nc.gpsimd.memset(payscal[:], 0.0)
            red = accp.tile([1, 512], f32, tag="acc")
            pe_tail.append(nc.tensor.matmul(red[0:1, 0:1], ones[:, 0:1],
                                            stv[:], start=True, stop=True))
            nc.vector.tensor_copy(payscal[0:1, 0:1], red[0:1, 0:1])
            red2 = accp.tile([1, 512], f32, tag="acc")
            pe_tail.append(nc.tensor.matmul(red2[0:1, 0:1], ones[:, 0:1],
                                            s1v[:], start=True, stop=True))
            nc.vector.tensor_copy(payscal[0:1, 1:2], red2[0:1, 0:1])
            nc.vector.tensor_reduce(payscal[0:1, 2:3], sxrow[0:1, :],
                                    axis=AXIS.X, op=ALU.add)
            nc.gpsimd.dma_start(pay2_d[0:1, :], payscal[0:1, 0:8])
            nc.gpsimd.collective_compute(
                "AllGather", ALU.bypass,
                replica_groups=[list(range(NCORES))],
                ins=[pay2_d.ap().opt()], outs=[ag2_d.ap().opt()])

            # ---- final math (identical on all cores) ----
            CC = KB // P
            S0 = persist.tile([P, CC], f32, tag="S0")
            s0t = persist.tile([P, CC], f32, tag="s0t")
            for ci in range(NCORES):
                tgt = S0 if ci == 0 else s0t
                nc.sync.dma_start(
                    tgt[:], ag_d[ci, 0:KB].rearrange("(c p) -> p c", p=P))
                if ci > 0:
                    nc.vector.tensor_add(S0[:], S0[:], s0t[:])
            for ci in range(NCORES):
                nc.sync.dma_start(
                    s1g_d.ap().rearrange("(k ci i) -> k ci i", k=K,
                                         ci=NCORES)[:, ci, :],
                    ag_d[ci, KB:KB + R].rearrange("(k i) -> k i", k=K))
                nc.sync.dma_start(
                    posg_d.ap().rearrange("(k ci i) -> k ci i", k=K,
                                          ci=NCORES)[:, ci, :],
                    ag_d[ci, KB + R:KB + 2 * R].rearrange(
                        "(k i) -> k i", k=K))
            S1A = persist.tile([P, CC], f32, tag="S1A")
            nc.sync.dma_start(
                S1A[:], s1g_d.ap().rearrange("(c p) -> p c", p=P))
            posA = persist.tile([P, CC], f32, tag="posA")
            nc.sync.dma_start(
                posA[:], posg_d.ap().rearrange("(c p) -> p c", p=P))
            stats = persist.tile([NCORES, 3], f32, tag="stats")
            nc.sync.dma_start(stats[:], ag2_d[:, 0:3])
            tps = accp.tile([1, 512], f32, tag="acc")
            pe_tail.append(nc.tensor.matmul(
                tps[0:1, 0:3], ones[0:NCORES, 0:1], stats[:],
                start=True, stop=True))
            tot = persist.tile([1, 4], f32, tag="tot")
            nc.vector.tensor_copy(tot[0:1, 0:3], tps[0:1, 0:3])

            d0 = persist.tile([P, CC], f32, tag="d0")
            nc.vector.scalar_tensor_tensor(
                d0[:], posA[:], -1.0, S0[:], op0=ALU.mult, op1=ALU.add)
            d1 = persist.tile([P, CC], f32, tag="d1")
            nc.vector.scalar_tensor_tensor(
                d1[:], posA[:], -1.0, S1A[:], op0=ALU.mult, op1=ALU.add)
            lp = persist.tile([P, CC], f32, tag="lp")
            tail_lns.append(nc.scalar.activation(lp[:], posA[:], FT_.Ln))
            la = persist.tile([P, CC], f32, tag="la")
            tail_lns.append(nc.scalar.activation(la[:], d0[:], FT_.Ln))
            lb = persist.tile([P, CC], f32, tag="lb")
            tail_lns.append(nc.scalar.activation(lb[:], d1[:], FT_.Ln))
            a0 = persist.tile([P, 1], f32, tag="a0")
            t0 = persist.tile([P, CC], f32, tag="t0")
            nc.vector.scalar_tensor_tensor(
                t0[:], la[:], -1.0, lp[:], op0=ALU.mult, op1=ALU.add,
                accum_out=a0[:])
            a1 = persist.tile([P, 1], f32, tag="a1")
            t1 = persist.tile([P, CC], f32, tag="t1")
            nc.vector.scalar_tensor_tensor(
                t1[:], lb[:], -1.0, lp[:], op0=ALU.mult, op1=ALU.add,
                accum_out=a1[:])
            a01 = persist.tile([P, 1], f32, tag="a01")
            nc.vector.tensor_add(a01[:], a0[:], a1[:])
            mip = accp.tile([1, 512], f32, tag="acc")
            pe_tail.append(nc.tensor.matmul(
                mip[0:1, 0:1], ones[:, 0:1], a01[:], start=True, stop=True))
            sc = persist.tile([1, 8], f32, tag="sc")
            nc.scalar.mul(sc[0:1, 1:2], mip[0:1, 0:1], 1.0 / (2.0 * B))
            nc.scalar.activation(sc[0:1, 5:6], tot[0:1, 2:3],
                                 FT_.Identity, bias=cb_bn[0:1, 0:1],
                                 scale=-1.0)
            nc.vector.reciprocal(sc[0:1, 5:6], sc[0:1, 5:6])
            nc.vector.reciprocal(sc[0:1, 6:7], tot[0:1, 2:3])
            nc.vector.tensor_sub(sc[0:1, 7:8], tot[0:1, 0:1], tot[0:1, 1:2])
            nc.vector.tensor_mul(sc[0:1, 3:4], sc[0:1, 7:8], sc[0:1, 5:6])
            nc.vector.tensor_mul(sc[0:1, 4:5], tot[0:1, 1:2], sc[0:1, 6:7])
            nc.vector.tensor_add(sc[0:1, 2:3], sc[0:1, 3:4], sc[0:1, 4:5])
            nc.vector.tensor_add(sc[0:1, 0:1], sc[0:1, 1:2], sc[0:1, 2:3])
            nc.sync.dma_start(out_d[0:1, :], sc[0:1, :])

        # -------- engine stream ordering --------
        from concourse.tile_rust import add_dep_helper
        nexp = len(exp_instrs)
        act_chain = (ln_h[0][:2] + nrm_acts + ln_h[0][2:] + ln_h[1]
                     + exp_instrs[:nexp // 2 + 1] + sig_h[0] + sig_h[1]
                     + ln_h[2] + ln_h[3] + exp_instrs[nexp // 2 + 1:]
                     + sig_h[2] + sig_h[3] + ln_h[4] + ln_h[5]
                     + sig_h[4] + sig_h[5] + ln_h[6] + ln_h[7]
                     + sig_h[6] + sig_h[7] + tail_lns)
        for prev, nxt in zip(act_chain, act_chain[1:]):
            add_dep_helper(nxt.ins, prev.ins, sync=False,
                           reason="act-table group ordering")
        pe_partB = []
        LAG = 2
        for cb in range(len(pe_S_cb)):
            for rb in range(RB):
                pe_partB += pe_S_cb[cb][2 * rb:2 * rb + 2]
                if rb >= LAG:
                    pe_partB += pe_C_cb[cb][2 * (rb - LAG):2 * (rb - LAG) + 2]
            for rb in range(RB - LAG, RB):
                pe_partB += pe_C_cb[cb][2 * rb:2 * rb + 2]
        half_pe = len(pe_partB) // 2
        pe_chain = (pe_norm + pe_sx[0] + pe_sx[1] + pe_partB[:half_pe]
                    + pe_sx[2] + pe_sx[3] + pe_partB[half_pe:]
                    + sum(pe_sx[4:], []) + pe_tail)
        for prev, nxt in zip(pe_chain, pe_chain[1:]):
            add_dep_helper(nxt.ins, prev.ins, sync=False,
                           reason="PE stream ordering")
        dve_chain = (dve_pre + dve_L[0] + dve_L[1] + dve_mid
                     + dve_col[0:3] + dve_L[2] + dve_L[3] + dve_col[3:]
                     + sum(dve_L[4:], []))
        for prev, nxt in zip(dve_chain, dve_chain[1:]):
            add_dep_helper(nxt.ins, prev.ins, sync=False,
                           reason="DVE stream ordering")

    return nc


_CACHE = {}


def _get_compiled():
    if "nc" not in _CACHE:
        nc = build()
        nc.compile()
        _CACHE["nc"] = nc
    return _CACHE["nc"]


def _make_in_maps(x, prob, u, z1, z2):
    x = np.asarray(x, np.float32)
    prob = np.asarray(prob, np.float32)
    u = np.asarray(u, np.float32)
    z1 = np.asarray(z1, np.float32)
    z2 = np.asarray(z2, np.float32)
    z2f = z2.reshape(K * B, D)
    z2t = np.ascontiguousarray(z2f.T)  # [64, 8192]
    in_maps = []
    for ci in range(NCORES):
        sl = slice(ci * BROWS, (ci + 1) * BROWS)
        z1s = z1[:, sl, :]                                   # [4,256,64]
        z1t = np.ascontiguousarray(
            z1s.transpose(2, 0, 1).reshape(D, R))            # [64,1024]
        z2ts = np.ascontiguousarray(np.concatenate(
            [z2t[:, k * B + ci * BROWS: k * B + (ci + 1) * BROWS]
             for k in range(K)], axis=1))                    # [64,1024]
        in_maps.append({
            "xb": np.ascontiguousarray(x[sl].reshape(P, FREE)).astype(
                ml_dtypes.bfloat16),
            "u": np.ascontiguousarray(u[sl].reshape(P, FREE)),
            "prob": np.ascontiguousarray(prob[sl].reshape(P, FREE)),
            "z1t": z1t,
            "z2t": z2t,
            "z2ts": z2ts,
        })
    return in_maps


def run(x, prob, u, z1, z2, trace=False, trace_kwargs=None):
    nc = _get_compiled()
    in_maps = _make_in_maps(x, prob, u, z1, z2)
    res = run_bass_kernel_spmd(nc, in_maps, core_ids=list(range(NCORES)),
                               trace=trace, **(trace_kwargs or {}))
    out = np.asarray(res.results[0]["out"], np.float32).reshape(-1)[:5]
    return out, res


def kernel(x, prob, u, z1, z2):
    out, _ = run(x, prob, u, z1, z2, trace=False)
    return out



# revision 16
# speedup vs baseline: 1.1215x; 1.1215x over previous
"""Trainium2 Bass kernel for nn_ADV_75574244541027 (gumbel-augmentation loss).

Self-contained: hardcodes shapes/sharding; builds one SPMD Bass program,
runs it on 8 NeuronCores via run_bass_kernel_spmd, returns the 5-scalar
output [aug_loss, mi, reg_aug, add, drop].

Structure (v2):
  part B (calc_I) first: sim-block matmuls -> exp -> PE column sums written
  in owner-permuted order -> ReduceScatter(add) fully overlapped with
  part A; each core then computes its local loss0/loss1 partial scalars.
  part A streams u (f32) + prob/x (bf16) in 20 tiles of 2000 columns,
  5 phases: 2 Ln passes (ACT), dt/w on DVE (bf16 2x), one Sigmoid pass
  (ACT, accum), x*sg on DVE (accum).  Each core outputs 4 partial scalars
  [st, s1, sx, L01]; the host sums the 8x4 partials and forms the final
  5 outputs (the gather/unshard step).
"""

import sys

for _p in ("/opt/trn_rl_repo", "/root/.axon_site/_ro/trn_rl_repo"):
    if _p not in sys.path:
        sys.path.insert(0, _p)

import numpy as np
import ml_dtypes

import concourse.bass as bass
import concourse.mybir as mybir
import concourse.bacc as bacc
import concourse.tile as tile
import concourse.bass_isa as bass_isa
from concourse.bass_utils import run_bass_kernel_spmd

# ---------------- problem constants ----------------
NCORES = 8
B, N, K, D = 2048, 20000, 4, 64
BIAS = 1e-4
C1 = 1.0 - 2.0 * BIAS          # (1 - 2*bias)
S5 = 5.0                       # 1/TAU == 1/TEMP == 5
P = 128
BROWS = B // NCORES            # 256 batch rows per core
FREE = BROWS * N // P          # 40000 free-dim elements per partition
FT = 2000                      # part-A tile width
NT = FREE // FT                # 20 tiles
NPH = 5                        # part-A phases (sigmoid batches)
TPH = NT // NPH                # 4 tiles per phase
R = K * BROWS                  # 1024 sim-matrix rows per core
RB = R // P                    # 8 row blocks
CBW = 1024                     # part-B column block width
KB = K * B                     # 8192 sim-matrix columns
NCB = KB // CBW                # 8 column blocks
ZQ = 512                       # z2-norm chunk width
NZQ = KB // ZQ                 # 16 chunks
BN = float(B) * float(N)

f32 = mybir.dt.float32
bf16 = mybir.dt.bfloat16
FT_ = mybir.ActivationFunctionType
ALU = mybir.AluOpType
AXIS = mybir.AxisListType


def build():
    nc = bacc.Bacc("TRN2", target_bir_lowering=False, debug=False,
                   num_devices=NCORES)

    x_d = nc.dram_tensor("xb", [P, FREE], bf16, kind="ExternalInput")
    u_d = nc.dram_tensor("u", [P, FREE], f32, kind="ExternalInput")
    p_d = nc.dram_tensor("prob", [P, FREE], bf16, kind="ExternalInput")
    z1t_d = nc.dram_tensor("z1t", [D, R], f32, kind="ExternalInput")
    z2t_d = nc.dram_tensor("z2t", [D, KB], f32, kind="ExternalInput")
    z2ts_d = nc.dram_tensor("z2ts", [D, R], f32, kind="ExternalInput")
    out_d = nc.dram_tensor("out", [1, 8], f32, kind="ExternalOutput")

    bn1_d = nc.dram_tensor("bn1", [R], f32)
    bns_d = nc.dram_tensor("bns", [R], f32)
    bdot_d = nc.dram_tensor("bdot", [R], f32)
    bn2_d = nc.dram_tensor("bn2", [KB], f32)
    bn2b_d = nc.dram_tensor("bn2b", [KB], f32)
    cs_d = nc.dram_tensor("cs", [1, KB], f32)          # permuted colsum
    rsc_d = nc.dram_tensor("rsc", [1, R], f32)         # ReduceScatter out

    # per-engine stream-order lists
    nrm_acts = []          # z1/z2s norm ln+exp
    z2_acts = []           # z2 full-norm ln+exp (per chunk pair)
    pos_exp = []           # pos exp
    exp_instrs = []        # part B exps (cb-major)
    lnA = [[] for _ in range(NT)]   # part A ln pairs per tile
    sigA = [None] * NT
    tail_lns = []          # lb1, lb0
    pe_norm = []
    pe_z2 = []
    pe_sx = [[] for _ in range(NT)]
    pe_SC = [[] for _ in range(NCB)]
    pe_tail = []
    dve_pre = []           # z1/z2s sq + dot
    dve_sqq = [[] for _ in range(NZQ)]
    dve_mid = []           # norm combines (m1/m2/lnpos/scale5)
    dve_nrmmul = [[] for _ in range(NZQ)]
    dve_L = [[] for _ in range(NT)]   # dt/w per tile
    dve_xs = [None] * NT
    dve_cst = [[] for _ in range(NCB)]
    dve_tail1 = []         # rs reduce, d1, t1
    dve_tail0 = []         # d0, t0, a01
    dve_fin = []           # st/s1 reduces, sxrow, sc copies

    with tile.TileContext(nc) as tc:
        from contextlib import ExitStack
        es = ExitStack()
        with es:
            persist = es.enter_context(tc.tile_pool(name="persist", bufs=1))

            ones = persist.tile([P, 1], f32, tag="ones")
            nc.gpsimd.memset(ones[:], 1.0)
            ones_b = persist.tile([P, 1], bf16, tag="ones_b")
            nc.gpsimd.memset(ones_b[:], 1.0)
            onesD = persist.tile([D, 1], f32, tag="onesD")
            nc.gpsimd.memset(onesD[:], 1.0)
            cb_hi = persist.tile([P, 1], f32, tag="cb_hi")
            nc.gpsimd.memset(cb_hi[:], 1.0 - BIAS)
            cb_lo = persist.tile([P, 1], f32, tag="cb_lo")
            nc.gpsimd.memset(cb_lo[:], BIAS)

            st_all = persist.tile([P, NT], f32, tag="st_all")
            s1_all = persist.tile([P, NT], f32, tag="s1_all")
            st2 = persist.tile([P, 2], f32, tag="st2")
            scale5 = persist.tile([P, RB], f32, tag="scale5")
            rn1 = persist.tile([P, RB], f32, tag="rn1")
            rn2s = persist.tile([P, RB], f32, tag="rn2s")
            pos_sb = persist.tile([P, RB], f32, tag="pos_sb")
            lnpos = persist.tile([P, RB], f32, tag="lnpos")
            rs_all = persist.tile([P, RB * NCB], f32, tag="rs_all")
            rs = persist.tile([P, RB], f32, tag="rs")
            S0loc = persist.tile([P, RB], f32, tag="S0loc")
            d0 = persist.tile([P, RB], f32, tag="d0")
            d1 = persist.tile([P, RB], f32, tag="d1")
            lb0 = persist.tile([P, RB], f32, tag="lb0")
            lb1 = persist.tile([P, RB], f32, tag="lb1")
            t0o = persist.tile([P, RB], f32, tag="t0o")
            t1o = persist.tile([P, RB], f32, tag="t1o")
            a0 = persist.tile([P, 1], f32, tag="a0")
            a1 = persist.tile([P, 1], f32, tag="a1")
            a01 = persist.tile([P, 1], f32, tag="a01")
            sxrow = persist.tile([1, 512], f32, tag="sxrow")
            sc = persist.tile([1, 8], f32, tag="sc")
            nc.gpsimd.memset(sc[:], 0.0)

            # ---- part B operands (permanent) ----
            pb = es.enter_context(tc.tile_pool(name="pb", bufs=1))
            z1t_sb = pb.tile([D, R], f32, tag="z1t")
            nc.gpsimd.dma_start(z1t_sb[:], z1t_d[:, :])
            z1tb = pb.tile([D, R], bf16, tag="z1tb")
            nc.vector.tensor_copy(z1tb[:], z1t_sb[:])
            z2nt = pb.tile([D, KB], bf16, tag="z2nt")
            z2ts_sb = pb.tile([D, R], f32, tag="z2ts")
            nc.gpsimd.dma_start(z2ts_sb[:], z2ts_d[:, :])

            psS = es.enter_context(
                tc.tile_pool(name="psS", bufs=2, space="PSUM"))
            psC = es.enter_context(
                tc.tile_pool(name="psC", bufs=1, space="PSUM"))
            accA = es.enter_context(
                tc.tile_pool(name="accA", bufs=1, space="PSUM"))
            accB = es.enter_context(
                tc.tile_pool(name="accB", bufs=1, space="PSUM"))

            def psum_colsums(dst_row, sq_tile, width, pe_list, dve_list):
                for c in range(0, width, 512):
                    cl = min(512, width - c)
                    pr = psS.tile([1, 512], f32, tag="S2")
                    pe_list.append(nc.tensor.matmul(
                        pr[0:1, 0:cl], onesD[:, 0:1], sq_tile[:, c:c + cl],
                        start=True, stop=True))
                    dve_list.append(nc.vector.tensor_copy(
                        dst_row[0:1, c:c + cl], pr[0:1, 0:cl]))

            zn = es.enter_context(tc.tile_pool(name="znorm", bufs=2))
            znA = es.enter_context(tc.tile_pool(name="znormA", bufs=1))
            # ---- z1 / z2-shard norms, diag -> pos ----
            sq1 = znA.tile([D, R], f32, tag="sq1")
            dve_pre.append(nc.vector.tensor_mul(sq1[:], z1t_sb[:], z1t_sb[:]))
            n21 = znA.tile([1, R], f32, tag="n21")
            psum_colsums(n21, sq1, R, pe_norm, dve_pre)
            nc.gpsimd.dma_start(bn1_d[:], n21[0:1, :])
            nc.gpsimd.dma_start(
                rn1[:], bn1_d.ap().rearrange("(c p) -> p c", p=P))
            nrm_acts.append(nc.scalar.activation(rn1[:], rn1[:], FT_.Ln))
            nrm_acts.append(nc.scalar.activation(
                rn1[:], rn1[:], FT_.Exp, scale=-0.5))
            dve_mid.append(nc.vector.tensor_scalar_mul(scale5[:], rn1[:], S5))

            sqs = znA.tile([D, R], f32, tag="sq1")
            dve_pre.append(nc.vector.tensor_mul(sqs[:], z2ts_sb[:], z2ts_sb[:]))
            n2sr = znA.tile([1, R], f32, tag="n21")
            psum_colsums(n2sr, sqs, R, pe_norm, dve_pre)
            nc.gpsimd.dma_start(bns_d[:], n2sr[0:1, :])
            nc.gpsimd.dma_start(
                rn2s[:], bns_d.ap().rearrange("(c p) -> p c", p=P))
            nrm_acts.append(nc.scalar.activation(rn2s[:], rn2s[:], FT_.Ln))
            nrm_acts.append(nc.scalar.activation(
                rn2s[:], rn2s[:], FT_.Exp, scale=-0.5))

            qd = znA.tile([D, R], f32, tag="sq1")
            dve_pre.append(nc.vector.tensor_mul(qd[:], z1t_sb[:], z2ts_sb[:]))
            dotr = znA.tile([1, R], f32, tag="n21")
            psum_colsums(dotr, qd, R, pe_norm, dve_pre)
            nc.gpsimd.dma_start(bdot_d[:], dotr[0:1, :])
            dots = persist.tile([P, RB], f32, tag="dots")
            nc.gpsimd.dma_start(
                dots[:], bdot_d.ap().rearrange("(c p) -> p c", p=P))
            m1 = persist.tile([P, RB], f32, tag="m1")
            dve_mid.append(nc.vector.tensor_mul(m1[:], dots[:], rn1[:]))
            m2 = persist.tile([P, RB], f32, tag="m2")
            dve_mid.append(nc.vector.tensor_mul(m2[:], m1[:], rn2s[:]))
            dve_mid.append(nc.vector.tensor_scalar_mul(lnpos[:], m2[:], S5))
            pos_exp.append(nc.scalar.activation(
                pos_sb[:], m2[:], FT_.Exp, scale=S5))

            # ---- normalize z2 in ZQ-wide chunks ----
            for q in range(NZQ):
                sl = slice(q * ZQ, (q + 1) * ZQ)
                z2q = zn.tile([D, ZQ], f32, tag="z2q")
                nc.sync.dma_start(z2q[:], z2t_d[:, sl])
                sqq = zn.tile([D, ZQ], f32, tag="sqq")
                dve_sqq[q].append(nc.vector.tensor_mul(sqq[:], z2q[:], z2q[:]))
                nrow = zn.tile([1, ZQ], f32, tag="nrow")
                psum_colsums(nrow, sqq, ZQ, pe_z2, dve_sqq[q])
                nc.gpsimd.dma_start(bn2_d[sl], nrow[0:1, :])
                nbq = persist.tile([P, ZQ // P], f32, tag=f"nbq{q}")
                nc.gpsimd.dma_start(
                    nbq[:], bn2_d[sl].rearrange("(c p) -> p c", p=P))
                z2_acts.append(nc.scalar.activation(nbq[:], nbq[:], FT_.Ln))
                z2_acts.append(nc.scalar.activation(
                    nbq[:], nbq[:], FT_.Exp, scale=-0.5))
                nc.gpsimd.dma_start(
                    bn2b_d[sl].rearrange("(c p) -> p c", p=P), nbq[:])
                bcq = zn.tile([D, ZQ], f32, tag="bcq")
                nc.gpsimd.dma_start(
                    bcq[:],
                    bn2b_d[sl].rearrange("(o n) -> o n", o=1).to_broadcast(
                        (D, ZQ)))
                z2q2 = zn.tile([D, ZQ], f32, tag="z2q2")
                nc.sync.dma_start(z2q2[:], z2t_d[:, sl])
                dve_nrmmul[q].append(nc.vector.tensor_mul(
                    z2nt[:, sl], z2q2[:], bcq[:]))

            # ---- part B main loop ----
            pbl = es.enter_context(tc.tile_pool(name="pbl", bufs=3))
            colp = es.enter_context(tc.tile_pool(name="colp", bufs=2))
            csv = cs_d[0:1, :].rearrange("p (o k i) -> p o k i", o=NCORES,
                                         k=K)
            for cb in range(NCB):
                cps = psC.tile([1, CBW], f32, tag="C")
                for rb in range(RB):
                    sps = psS.tile([P, CBW], f32, tag="S2")
                    for half in range(2):
                        pe_SC[cb].append(nc.tensor.matmul(
                            sps[:, half * 512:(half + 1) * 512],
                            z1tb[:, rb * P:(rb + 1) * P],
                            z2nt[:, cb * CBW + half * 512:
                                 cb * CBW + (half + 1) * 512],
                            start=True, stop=True))
                    et = pbl.tile([P, CBW], bf16, tag="E")
                    exp_instrs.append(nc.scalar.activation(
                        et[:], sps[:], FT_.Exp,
                        scale=scale5[:, rb:rb + 1],
                        accum_out=rs_all[:, rb * NCB + cb:
                                         rb * NCB + cb + 1]))
                    for half in range(2):
                        pe_SC[cb].append(nc.tensor.matmul(
                            cps[0:1, half * 512:(half + 1) * 512],
                            ones_b[:, 0:1],
                            et[:, half * 512:(half + 1) * 512],
                            start=(rb == 0), stop=(rb == RB - 1)))
                cst = colp.tile([1, CBW], f32, tag="cst")
                dve_cst[cb].append(nc.vector.tensor_copy(
                    cst[0:1, :], cps[0:1, :]))
                # permuted write: col c=cb*CBW+j -> owner*R + k*BROWS + i
                # (gpsimd ring: keeps the sync ring free for part-A streams)
                dst = csv[0:1, 4 * (cb % 2):4 * (cb % 2) + 4, cb // 2, :]
                nc.gpsimd.dma_start(
                    dst, cst[0:1, :].rearrange("p (q i) -> p q i", q=4))

            # ReduceScatter of permuted colsums -> own diag columns
            nc.gpsimd.collective_compute(
                "ReduceScatter", ALU.add,
                replica_groups=[list(range(NCORES))],
                ins=[cs_d.ap().opt()], outs=[rsc_d.ap().opt()])
            nc.gpsimd.dma_start(
                S0loc[:], rsc_d[0, 0:R].rearrange("(rb p) -> p rb", p=P))

            # local loss1 partial (rowsum-based)
            dve_tail1.append(nc.vector.tensor_reduce(
                rs[:], rs_all[:].rearrange("p (rb cb) -> p rb cb", cb=NCB),
                axis=AXIS.X, op=ALU.add))
            dve_tail1.append(nc.vector.scalar_tensor_tensor(
                d1[:], pos_sb[:], -1.0, rs[:], op0=ALU.mult, op1=ALU.add))
            tail_lns.append(nc.scalar.activation(lb1[:], d1[:], FT_.Ln))
            dve_tail1.append(nc.vector.scalar_tensor_tensor(
                t1o[:], lb1[:], -1.0, lnpos[:], op0=ALU.mult, op1=ALU.add,
                accum_out=a1[:]))

            # local loss0 partial (global colsum slice via RS)
            dve_tail0.append(nc.vector.scalar_tensor_tensor(
                d0[:], pos_sb[:], -1.0, S0loc[:], op0=ALU.mult, op1=ALU.add))
            tail_lns.append(nc.scalar.activation(lb0[:], d0[:], FT_.Ln))
            dve_tail0.append(nc.vector.scalar_tensor_tensor(
                t0o[:], lb0[:], -1.0, lnpos[:], op0=ALU.mult, op1=ALU.add,
                accum_out=a0[:]))
            dve_tail0.append(nc.vector.tensor_add(a01[:], a0[:], a1[:]))

            # ---- part A ----
            plu = es.enter_context(tc.tile_pool(name="plu", bufs=3))
            plp = es.enter_context(tc.tile_pool(name="plp", bufs=3))
            psx = es.enter_context(tc.tile_pool(name="psx", bufs=7))
            plt = es.enter_context(tc.tile_pool(name="plt", bufs=2))
            wpool = es.enter_context(tc.tile_pool(name="wpool", bufs=2))
            pscr = es.enter_context(tc.tile_pool(name="pscr", bufs=2))

            acc_sx = accA.tile([1, 512], f32, tag="acc")
            xt_tiles = [None] * NT
            for ph in range(NPH):
                w_sb = wpool.tile([P, TPH * FT], bf16, tag="w")
                for ti in range(TPH):
                    t = ph * TPH + ti
                    sl = slice(t * FT, (t + 1) * FT)
                    wsl = slice(ti * FT, (ti + 1) * FT)
                    ut = plu.tile([P, FT], f32, tag="u")
                    nc.sync.dma_start(ut[:], u_d[:, sl])
                    pt = plp.tile([P, FT], bf16, tag="p")
                    nc.sync.dma_start(pt[:], p_d[:, sl])
                    xt = psx.tile([P, FT], bf16, tag="x")
                    nc.sync.dma_start(xt[:], x_d[:, sl])
                    xt_tiles[t] = xt
                    l1 = plt.tile([P, FT], bf16, tag="l1")
                    lnA[t].append(nc.scalar.activation(
                        l1[:], ut[:], FT_.Ln, bias=cb_hi[:, 0:1],
                        scale=-C1))
                    l2 = plt.tile([P, FT], bf16, tag="l2")
                    lnA[t].append(nc.scalar.activation(
                        l2[:], ut[:], FT_.Ln, bias=cb_lo[:, 0:1], scale=C1))
                    dt = plt.tile([P, FT], bf16, tag="dt")
                    dve_L[t].append(nc.vector.scalar_tensor_tensor(
                        dt[:], l1[:], -1.0, l2[:], op0=ALU.mult,
                        op1=ALU.add))
                    dve_L[t].append(nc.vector.scalar_tensor_tensor(
                        w_sb[:, wsl], dt[:], 1.0, pt[:],
                        op0=ALU.mult, op1=ALU.subtract))
                    last = (t == NT - 1)
                    for c in range(0, FT, 512):
                        cl = min(512, FT - c)
                        pe_sx[t].append(nc.tensor.matmul(
                            acc_sx[0:1, 0:cl], ones_b[:, 0:1],
                            xt[:, c:c + cl],
                            start=(t == 0 and c == 0),
                            stop=(last and c + 512 >= FT)))
                for ti in range(TPH):
                    t = ph * TPH + ti
                    wsl = slice(ti * FT, (ti + 1) * FT)
                    sg = pscr.tile([P, FT], bf16, tag="sg")
                    sigA[t] = nc.scalar.activation(
                        sg[:], w_sb[:, wsl], FT_.Sigmoid, scale=S5,
                        accum_out=st_all[:, t:t + 1])
                    xs = pscr.tile([P, FT], bf16, tag="xs")
                    dve_xs[t] = nc.vector.scalar_tensor_tensor(
                        xs[:], sg[:], 1.0, xt_tiles[t][:],
                        op0=ALU.mult, op1=ALU.mult,
                        accum_out=s1_all[:, t:t + 1])

            # ---- partial-scalar assembly ----
            dve_fin.append(nc.vector.tensor_reduce(
                st2[:, 0:1], st_all[:], axis=AXIS.X, op=ALU.add))
            dve_fin.append(nc.vector.tensor_reduce(
                st2[:, 1:2], s1_all[:], axis=AXIS.X, op=ALU.add))
            dve_fin.append(nc.vector.tensor_copy(sxrow[0:1, :],
                                                 acc_sx[0:1, :]))
            mip = accB.tile([1, 512], f32, tag="acc")
            pe_tail.append(nc.tensor.matmul(
                mip[0:1, 0:1], ones[:, 0:1], a01[:], start=True, stop=True))
            red = accB.tile([1, 512], f32, tag="acc")
            pe_tail.append(nc.tensor.matmul(
                red[0:1, 0:2], ones[:, 0:1], st2[:], start=True, stop=True))
            dve_fin.append(nc.vector.tensor_copy(sc[0:1, 3:4],
                                                 mip[0:1, 0:1]))
            dve_fin.append(nc.vector.tensor_copy(sc[0:1, 0:2],
                                                 red[0:1, 0:2]))
            dve_fin.append(nc.vector.tensor_reduce(
                sc[0:1, 2:3], sxrow[0:1, :], axis=AXIS.X, op=ALU.add))
            nc.sync.dma_start(out_d[0:1, :], sc[0:1, :])

        # -------- engine stream ordering --------
        from concourse.tile_rust import add_dep_helper

        def chain(lst, reason):
            for prev, nxt in zip(lst, lst[1:]):
                add_dep_helper(nxt.ins, prev.ins, sync=False, reason=reason)

        def interleave(a, b, na, nb):
            """merge a,b taking na of a then nb of b, repeating"""
            out = []
            ia = ib = 0
            while ia < len(a) or ib < len(b):
                out += a[ia:ia + na]; ia += na
                out += b[ib:ib + nb]; ib += nb
            return out

        # ACT: ln_exp table [norms + part-A lns + part-B exps] with one
        # sigmoid batch per phase (2 table switches per phase)
        act = []
        act += lnA[0]
        act += nrm_acts
        act += lnA[1] + pos_exp
        act += z2_acts[:8]
        act += lnA[2] + z2_acts[8:16]
        act += z2_acts[16:] + lnA[3]
        act += [sigA[0], sigA[1], sigA[2], sigA[3]]
        # remaining windows: 16 lns each; exps split 8/16/20/20
        esplit = [0, 8, 24, 44, 64]
        for ph in range(1, NPH):
            lns = sum((lnA[t] for t in range(ph * TPH, (ph + 1) * TPH)), [])
            exps = exp_instrs[esplit[ph - 1]:esplit[ph]]
            act += interleave(lns, exps, 2, 2)
            if ph == NPH - 1:
                act += tail_lns
            act += [sigA[t] for t in range(ph * TPH, (ph + 1) * TPH)]
        chain(act, "ACT stream ordering")

        # PE
        pe = []
        pe += pe_norm
        pe += pe_z2[:8]          # z2 chunks q0..q3
        pe += pe_sx[0] + pe_sx[1]
        pe += pe_z2[8:]
        pe += pe_sx[2] + pe_sx[3]
        # S/C per cb with LAG-2 C interleave, spread between xsum groups;
        # pe_SC[cb] holds per rb: [S0,S1,C0,C1] quadruples in rb order
        def sc_order(cb):
            LAG = 2
            out = []
            for rb in range(RB):
                out += pe_SC[cb][4 * rb:4 * rb + 2]
                if rb >= LAG:
                    out += pe_SC[cb][4 * (rb - LAG) + 2:4 * (rb - LAG) + 4]
            for rb in range(RB - LAG, RB):
                out += pe_SC[cb][4 * rb + 2:4 * rb + 4]
            return out
        nxt_t = 4
        for cb in range(NCB):
            pe += sc_order(cb)
            for t in range(nxt_t, min(nxt_t + 2, NT)):
                pe += pe_sx[t]
            nxt_t = min(nxt_t + 2, NT)
        for t in range(nxt_t, NT):
            pe += pe_sx[t]
        pe += pe_tail
        chain(pe, "PE stream ordering")

        # DVE: z-norm groups interleaved with first part-A tiles so that
        # neither ACT window blocks on a later DVE group (release edges)
        dve = []
        dve += dve_pre
        dve += sum(dve_sqq[:4], [])
        dve += dve_mid
        dve += sum(dve_sqq[4:8], []) + sum(dve_nrmmul[:4], [])
        dve += dve_L[0]
        dve += sum(dve_sqq[8:12], []) + sum(dve_nrmmul[4:8], [])
        dve += dve_L[1]
        dve += sum(dve_sqq[12:], []) + sum(dve_nrmmul[8:12], [])
        dve += dve_L[2]
        dve += sum(dve_nrmmul[12:], [])
        dve += dve_L[3]
        # per phase ph>=1: dt/w interleaved with xsg of previous phase;
        # colsum copies of the cbs whose exps land in that ACT window
        csplit = [0, 1, 3, 5, 8]
        for ph in range(1, NPH):
            dve += [dve_xs[(ph - 1) * TPH + ti] for ti in range(TPH)]
            for ti in range(TPH):
                dve += dve_L[ph * TPH + ti]
            dve += sum(dve_cst[csplit[ph - 1]:csplit[ph]], [])
            if ph == NPH - 1:
                dve += dve_tail1 + dve_tail0
        # last phase sigmoid products + finals
        dve += [dve_xs[(NPH - 1) * TPH + ti] for ti in range(TPH)]
        dve += dve_fin
        # d1/t1 depend on lb1 (ACT) etc.; data deps handle exact timing
        chain(dve, "DVE stream ordering")

    return nc


_CACHE = {}


def _get_compiled():
    if "nc" not in _CACHE:
        nc = build()
        nc.compile()
        _CACHE["nc"] = nc
    return _CACHE["nc"]


def _make_in_maps(x, prob, u, z1, z2):
    x = np.asarray(x, np.float32)
    prob = np.asarray(prob, np.float32)
    u = np.asarray(u, np.float32)
    z1 = np.asarray(z1, np.float32)
    z2 = np.asarray(z2, np.float32)
    z2f = z2.reshape(K * B, D)
    z2t = np.ascontiguousarray(z2f.T)  # [64, 8192]
    in_maps = []
    for ci in range(NCORES):
        sl = slice(ci * BROWS, (ci + 1) * BROWS)
        z1s = z1[:, sl, :]                                   # [4,256,64]
        z1t = np.ascontiguousarray(
            z1s.transpose(2, 0, 1).reshape(D, R))            # [64,1024]
        z2ts = np.ascontiguousarray(np.concatenate(
            [z2t[:, k * B + ci * BROWS: k * B + (ci + 1) * BROWS]
             for k in range(K)], axis=1))                    # [64,1024]
        in_maps.append({
            "xb": np.ascontiguousarray(x[sl].reshape(P, FREE)).astype(
                ml_dtypes.bfloat16),
            "u": np.ascontiguousarray(u[sl].reshape(P, FREE)),
            "prob": np.ascontiguousarray(prob[sl].reshape(P, FREE)).astype(
                ml_dtypes.bfloat16),
            "z1t": z1t,
            "z2t": z2t,
            "z2ts": z2ts,
        })
    return in_maps


def run(x, prob, u, z1, z2, trace=False, trace_kwargs=None):
    nc = _get_compiled()
    in_maps = _make_in_maps(x, prob, u, z1, z2)
    res = run_bass_kernel_spmd(nc, in_maps, core_ids=list(range(NCORES)),
                               trace=trace, **(trace_kwargs or {}))
    # gather/unshard: sum per-core partial scalars, form final outputs
    parts = np.zeros(4, np.float64)
    for ci in range(NCORES):
        parts += np.asarray(res.results[ci]["out"],
                            np.float32).reshape(-1)[:4].astype(np.float64)
    st, s1, sx, L01 = parts
    add = (st - s1) / (BN - sx)
    drop = s1 / sx
    reg = add + drop
    mi = L01 / (2.0 * B)
    aug = mi + reg
    out = np.array([aug, mi, reg, add, drop], np.float32)
    return out, res


def kernel(x, prob, u, z1, z2):
    out, _ = run(x, prob, u, z1, z2, trace=False)
    return out
